# revision 4
# baseline (speedup 1.0000x reference)
"""CosineSimCodebook (VQ) kernel for 8 Trainium2 NeuronCores.

Reference computes:
    dist = l2norm(x.flatten) @ l2norm(embed).T   # [1, B*N, C]
    ind  = argmax(dist, -1)
    quant = embed[0][ind]            # gather from UNNORMALIZED embed

Key simplification: row-wise l2norm of x scales each dist row by a
positive constant -> argmax invariant, so x need not be normalized.
embed is already unit-norm at init (reference re-normalizes it, which
perturbs columns only by ~1e-7 relative -> argmax ties at that scale are
below the fp32 matmul noise floor anyway), so the kernel computes raw
dist = x @ embed.T.

Precision: the PE runs bf16 at 1 cyc/row vs fp32 at 4. We split each
fp32 operand into bf16 hi+lo on the host (x = hi + lo exactly to
~2^-17 rel) and compute hi*hi + lo*hi + hi*lo with fp32 PSUM
accumulation -> max error ~5e-7, on par with the native fp32 matmul
path, at 3/4 the cost.

Sharding: tokens (B*N = 16384) split across 8 cores, 2048 each;
codebook replicated.
"""

import numpy as np
import ml_dtypes

import concourse.bacc as bacc
import concourse.bass as bass
import concourse.mybir as mybir
from concourse.tile import TileContext
from concourse.bass_utils import run_bass_kernel_spmd

# problem shape (hardcoded per contract)
B, N, D, C = 8, 2048, 256, 16384
N_CORES = 8
T = (B * N) // N_CORES          # tokens per core = 2048
P = 128                          # tokens per block (SBUF partitions)
N_BLOCKS = T // P                # 16
KCH = D // 128                   # contraction chunks = 2
QSIZE = 4096                     # codes per argmax quarter
N_QUARTERS = C // QSIZE          # 4
NCH = QSIZE // 512               # psum chunks per quarter = 8

_compiled = None


def _build(reps=1):
    f32 = mybir.dt.float32
    bf16 = mybir.dt.bfloat16
    nc = bacc.Bacc(None)

    xt_hi = nc.dram_tensor("xt_hi", [P, KCH, T], bf16, kind="ExternalInput")
    xt_lo = nc.dram_tensor("xt_lo", [P, KCH, T], bf16, kind="ExternalInput")
    et_hi = nc.dram_tensor("et_hi", [P, KCH, C], bf16, kind="ExternalInput")
    et_lo = nc.dram_tensor("et_lo", [P, KCH, C], bf16, kind="ExternalInput")
    embed_g = nc.dram_tensor("embed_g", [C, D], f32, kind="ExternalInput")
    quant = nc.dram_tensor("quant", [T, D], f32, kind="ExternalOutput")
    eind = nc.dram_tensor("eind", [T, 1], mybir.dt.int32, kind="ExternalOutput")

    with TileContext(nc) as tc:
        with (
            tc.tile_pool(name="inp", bufs=1) as inp,
            tc.tile_pool(name="dist", bufs=2) as distp,
            tc.tile_pool(name="small", bufs=2) as small,
            tc.tile_pool(name="outp", bufs=2) as outp,
            tc.tile_pool(name="psum", bufs=8, space="PSUM") as psum,
        ):
            xh = inp.tile([P, KCH, T], bf16, tag="xh")
            xl = inp.tile([P, KCH, T], bf16, tag="xl")
            eh = inp.tile([P, KCH, C], bf16, tag="eh")
            el = inp.tile([P, KCH, C], bf16, tag="el")
            nc.sync.dma_start(xh, xt_hi[:, :, :])
            nc.sync.dma_start(xl, xt_lo[:, :, :])
            # split the big embed loads so they spread across DMA queues
            for i in range(4):
                sl = slice(i * (C // 4), (i + 1) * (C // 4))
                nc.sync.dma_start(eh[:, :, sl], et_hi[:, :, sl])
                nc.sync.dma_start(el[:, :, sl], et_lo[:, :, sl])

            for _rep in range(reps):
              for blk in range(N_BLOCKS):
                tsl = slice(blk * P, (blk + 1) * P)
                run_max = small.tile([P, 1], f32, tag="run_max")
                run_idx = small.tile([P, 1], f32, tag="run_idx")
                for q in range(N_QUARTERS):
                    dist = distp.tile([P, QSIZE], f32, tag="dist")
                    for ch in range(NCH):
                        c0 = q * QSIZE + ch * 512
                        csl = slice(c0, c0 + 512)
                        ps = psum.tile([P, 512], f32, tag="ps")
                        mm = 0
                        for a, b_ in ((xh, eh), (xl, eh), (xh, el)):
                            for k in range(KCH):
                                nc.tensor.matmul(
                                    ps,
                                    a[:, k, tsl],
                                    b_[:, k, csl],
                                    start=(mm == 0),
                                    stop=(mm == 3 * KCH - 1),
                                )
                                mm += 1
                        nc.scalar.copy(dist[:, ch * 512 : (ch + 1) * 512], ps)
                    mx8 = small.tile([P, 8], f32, tag="mx8")
                    ix8 = small.tile([P, 8], mybir.dt.uint32, tag="ix8")
                    nc.vector.max(out=mx8, in_=dist)
                    nc.vector.max_index(out=ix8, in_max=mx8, in_values=dist)
                    idxf = small.tile([P, 1], f32, tag="idxf")
                    nc.vector.tensor_copy(idxf, ix8[:, :1])
                    if q == 0:
                        nc.vector.tensor_copy(run_max, mx8[:, :1])
                        nc.vector.tensor_copy(run_idx, idxf)
                    else:
                        nc.vector.tensor_scalar_add(idxf, idxf, float(q * QSIZE))
                        msk = small.tile([P, 1], mybir.dt.uint32, tag="msk")
                        nc.vector.tensor_tensor(
                            out=msk, in0=mx8[:, :1], in1=run_max,
                            op=mybir.AluOpType.is_gt,
                        )
                        nc.vector.copy_predicated(run_max, msk, mx8[:, :1])
                        nc.vector.copy_predicated(run_idx, msk, idxf)
                idx_i = outp.tile([P, 1], mybir.dt.int32, tag="idx_i")
                nc.vector.tensor_copy(idx_i, run_idx)
                qt = outp.tile([P, D], f32, tag="qt")
                nc.gpsimd.indirect_dma_start(
                    out=qt,
                    out_offset=None,
                    in_=embed_g[:, :],
                    in_offset=bass.IndirectOffsetOnAxis(ap=idx_i[:, :1], axis=0),
                )
                nc.sync.dma_start(quant[tsl, :], qt)
                nc.sync.dma_start(eind[tsl, :], idx_i)
    nc.compile()
    return nc


def _split_bf16(a):
    hi = a.astype(ml_dtypes.bfloat16)
    lo = (a - hi.astype(np.float32)).astype(ml_dtypes.bfloat16)
    return hi, lo


def _tposed(a2d, cols):
    # [cols, D=256] f32 -> bf16 hi/lo in [128, 2, cols] layout (d = k*128 + p)
    at = np.ascontiguousarray(a2d.T)            # [256, cols]
    hi, lo = _split_bf16(at)
    return (
        np.ascontiguousarray(hi.reshape(KCH, P, cols).transpose(1, 0, 2)),
        np.ascontiguousarray(lo.reshape(KCH, P, cols).transpose(1, 0, 2)),
    )


def kernel(x, embed, _trace=False, _tmpdir=None):
    global _compiled
    x = np.asarray(x, dtype=np.float32)
    embed = np.asarray(embed, dtype=np.float32)
    if _compiled is None:
        _compiled = _build()
    nc = _compiled

    emb2d = np.ascontiguousarray(embed.reshape(C, D))
    eh, el = _tposed(emb2d, C)

    flat = x.reshape(B * N, D)
    in_maps = []
    for c in range(N_CORES):
        shard = flat[c * T : (c + 1) * T]
        xh, xl = _tposed(shard, T)
        in_maps.append({
            "xt_hi": xh, "xt_lo": xl,
            "et_hi": eh, "et_lo": el,
            "embed_g": emb2d,
        })

    res = run_bass_kernel_spmd(
        nc, in_maps, list(range(N_CORES)), trace=_trace, tmpdir=_tmpdir
    )
    quant = np.concatenate(
        [res.results[c]["quant"][None] for c in range(N_CORES)], axis=0
    ).reshape(B, N, D)
    eind = np.concatenate(
        [res.results[c]["eind"].reshape(-1)[None] for c in range(N_CORES)], axis=0
    ).reshape(B, N).astype(np.int32)
    if _trace:
        kernel._last_exec_ns = res.exec_time_ns
    return quant, eind


# revision 13
# speedup vs baseline: 2.2435x; 2.2435x over previous
"""CosineSimCodebook (VQ) kernel for 8 Trainium2 NeuronCores.

Reference computes:
    dist = l2norm(x.flatten) @ l2norm(embed).T   # [1, B*N, C]
    ind  = argmax(dist, -1)
    quant = embed[0][ind]            # gather from UNNORMALIZED embed

Key simplification: row-wise l2norm of x scales each dist row by a
positive constant -> argmax invariant, so x need not be normalized.
embed is already unit-norm at init (reference re-normalizes it, which
perturbs columns only by ~1e-7 relative -> argmax ties at that scale are
below the fp32 matmul noise floor anyway), so the kernel computes raw
dist = x @ embed.T.

Precision: the PE runs bf16 at 1 cyc/row vs fp32 at 4. We split each
fp32 operand into bf16 hi+lo on the host (x = hi + lo exactly to
~2^-17 rel) and compute hi*hi + lo*hi + hi*lo with fp32 PSUM
accumulation -> max error ~5e-7, on par with the native fp32 matmul
path, at 3/4 the cost.

Sharding: tokens (B*N = 16384) split across 8 cores, 2048 each;
codebook replicated.
"""

import numpy as np
import ml_dtypes

import concourse.bacc as bacc
import concourse.bass as bass
import concourse.mybir as mybir
from concourse.tile import TileContext
from concourse.bass_utils import run_bass_kernel_spmd

# problem shape (hardcoded per contract)
B, N, D, C = 8, 2048, 256, 16384
N_CORES = 8
T = (B * N) // N_CORES          # tokens per core = 2048
P = 128                          # tokens per block (SBUF partitions)
N_BLOCKS = T // P                # 16
KCH = D // 128                   # contraction chunks = 2
QSIZE = 4096                     # codes per argmax quarter
N_QUARTERS = C // QSIZE          # 4
NCH = QSIZE // 512               # psum chunks per quarter = 8

_compiled = None


def _build(reps=1, qsize=2048, n_terms=3, dist_bufs=4, psum_bufs=8, gather=True, dve_slice=None, cw=512):
    # dve_slice/n_terms < full are TIMING-ONLY experiment knobs (wrong output)
    f32 = mybir.dt.float32
    bf16 = mybir.dt.bfloat16
    n_quarters = C // qsize
    nch = qsize // 512
    if dve_slice is None:
        dve_slice = qsize
    nc = bacc.Bacc(None)

    xt_hi = nc.dram_tensor("xt_hi", [P, KCH, T], bf16, kind="ExternalInput")
    xt_lo = nc.dram_tensor("xt_lo", [P, KCH, T], bf16, kind="ExternalInput")
    et_hi = nc.dram_tensor("et_hi", [P, KCH, C], bf16, kind="ExternalInput")
    et_lo = nc.dram_tensor("et_lo", [P, KCH, C], bf16, kind="ExternalInput")
    embed_g = nc.dram_tensor("embed_g", [C, D], f32, kind="ExternalInput")
    quant = nc.dram_tensor("quant", [T, D], f32, kind="ExternalOutput")
    eind = nc.dram_tensor("eind", [T, 1], mybir.dt.int32, kind="ExternalOutput")

    with TileContext(nc) as tc:
        with (
            tc.tile_pool(name="inp", bufs=1) as inp,
            tc.tile_pool(name="dist", bufs=dist_bufs) as distp,
            tc.tile_pool(name="small", bufs=2) as small,
            tc.tile_pool(name="outp", bufs=2) as outp,
            tc.tile_pool(name="psum", bufs=psum_bufs, space="PSUM") as psum,
        ):
            xh = inp.tile([P, KCH, T], bf16, tag="xh")
            xl = inp.tile([P, KCH, T], bf16, tag="xl")
            nc.sync.dma_start(xh, xt_hi[:, :, :])
            nc.sync.dma_start(xl, xt_lo[:, :, :])
            # per-quarter embed tiles: first matmuls start as soon as the
            # first quarter lands instead of waiting for the full 16 MB
            ehq, elq = [], []
            for i in range(n_quarters):
                sl = slice(i * qsize, (i + 1) * qsize)
                ehi = inp.tile([P, KCH, qsize], bf16, tag=f"eh{i}")
                eli = inp.tile([P, KCH, qsize], bf16, tag=f"el{i}")
                nc.sync.dma_start(ehi, et_hi[:, :, sl])
                nc.sync.dma_start(eli, et_lo[:, :, sl])
                ehq.append(ehi)
                elq.append(eli)

            # constant per-quarter global index offsets [P, n_quarters]
            qoff = inp.tile([P, n_quarters], f32, tag="qoff")
            for q in range(n_quarters):
                nc.vector.memset(qoff[:, q : q + 1], float(q * qsize))

            for _rep in range(reps):
              for blk in range(N_BLOCKS):
                tsl = slice(blk * P, (blk + 1) * P)
                # per-quarter top-8 values/indices staged side by side
                mxall = small.tile([P, n_quarters, 8], f32, tag="mxall")
                ixall = small.tile([P, n_quarters, 8], mybir.dt.uint32,
                                   tag="ixall")
                for q in range(n_quarters):
                    eh, el = ehq[q], elq[q]
                    dist = distp.tile([P, qsize], f32, tag="dist")
                    for cb in range(qsize // cw):
                        ps = psum.tile([P, cw], f32, tag="ps")
                        for sub in range(cw // 512):
                            c0 = cb * cw + sub * 512
                            csl = slice(c0, c0 + 512)
                            mm = 0
                            terms = ((xh, eh), (xl, eh), (xh, el))[:n_terms]
                            for a, b_ in terms:
                                for k in range(KCH):
                                    nc.tensor.matmul(
                                        ps[:, sub * 512 : (sub + 1) * 512],
                                        a[:, k, tsl],
                                        b_[:, k, csl],
                                        start=(mm == 0),
                                        stop=(mm == len(terms) * KCH - 1),
                                    )
                                    mm += 1
                        nc.scalar.copy(dist[:, cb * cw : (cb + 1) * cw], ps)
                    nc.vector.max(out=mxall[:, q, :], in_=dist[:, :dve_slice])
                    nc.vector.max_index(out=ixall[:, q, :],
                                        in_max=mxall[:, q, :],
                                        in_values=dist[:, :dve_slice])
                # merge across quarters arithmetically (no serial chain):
                # gmax = max_q mx[q]; idx = sum_q (mx[q]==gmax)*(ix[q]+q*qsize)
                tops = mxall[:, :, 0]          # [P, n_quarters] strided view
                gmax = small.tile([P, 1], f32, tag="gmax")
                nc.vector.reduce_max(out=gmax, in_=tops,
                                     axis=mybir.AxisListType.X)
                eqq = small.tile([P, n_quarters], f32, tag="eqq")
                nc.vector.tensor_tensor(
                    out=eqq, in0=tops,
                    in1=gmax.to_broadcast([P, n_quarters]),
                    op=mybir.AluOpType.is_equal,
                )
                idq = small.tile([P, n_quarters], f32, tag="idq")
                nc.vector.tensor_tensor(out=idq, in0=ixall[:, :, 0], in1=qoff,
                                        op=mybir.AluOpType.add)
                nc.vector.tensor_tensor(out=idq, in0=idq, in1=eqq,
                                        op=mybir.AluOpType.mult)
                idxf = small.tile([P, 1], f32, tag="idxf")
                nc.vector.reduce_sum(out=idxf, in_=idq,
                                     axis=mybir.AxisListType.X)
                idx_i = outp.tile([P, 1], mybir.dt.int32, tag="idx_i")
                nc.vector.tensor_copy(idx_i, idxf)
                if gather:
                    qt = outp.tile([P, D], f32, tag="qt")
                    nc.gpsimd.indirect_dma_start(
                        out=qt,
                        out_offset=None,
                        in_=embed_g[:, :],
                        in_offset=bass.IndirectOffsetOnAxis(ap=idx_i[:, :1], axis=0),
                    )
                    nc.sync.dma_start(quant[tsl, :], qt)
                nc.sync.dma_start(eind[tsl, :], idx_i)
    nc.compile()
    return nc


def _split_bf16(a):
    hi = a.astype(ml_dtypes.bfloat16)
    lo = (a - hi.astype(np.float32)).astype(ml_dtypes.bfloat16)
    return hi, lo


def _tposed(a2d, cols):
    # [cols, D=256] f32 -> bf16 hi/lo in [128, 2, cols] layout (d = k*128 + p)
    at = np.ascontiguousarray(a2d.T)            # [256, cols]
    hi, lo = _split_bf16(at)
    return (
        np.ascontiguousarray(hi.reshape(KCH, P, cols).transpose(1, 0, 2)),
        np.ascontiguousarray(lo.reshape(KCH, P, cols).transpose(1, 0, 2)),
    )


def kernel(x, embed, _trace=False, _tmpdir=None):
    global _compiled
    x = np.asarray(x, dtype=np.float32)
    embed = np.asarray(embed, dtype=np.float32)
    if _compiled is None:
        _compiled = _build()
    nc = _compiled

    emb2d = np.ascontiguousarray(embed.reshape(C, D))
    eh, el = _tposed(emb2d, C)

    flat = x.reshape(B * N, D)
    in_maps = []
    for c in range(N_CORES):
        shard = flat[c * T : (c + 1) * T]
        xh, xl = _tposed(shard, T)
        in_maps.append({
            "xt_hi": xh, "xt_lo": xl,
            "et_hi": eh, "et_lo": el,
            "embed_g": emb2d,
        })

    res = run_bass_kernel_spmd(
        nc, in_maps, list(range(N_CORES)), trace=_trace, tmpdir=_tmpdir
    )
    quant = np.concatenate(
        [res.results[c]["quant"][None] for c in range(N_CORES)], axis=0
    ).reshape(B, N, D)
    eind = np.concatenate(
        [res.results[c]["eind"].reshape(-1)[None] for c in range(N_CORES)], axis=0
    ).reshape(B, N).astype(np.int32)
    if _trace:
        kernel._last_exec_ns = res.exec_time_ns
    return quant, eind
